# revision 59
# baseline (speedup 1.0000x reference)
"""Beran survival estimator (nn_Beran) — Trainium2 Bass kernel.

kernel(**inputs) takes the FULL inputs (c_p [16,256,8] f32, c_in [8192,16] int,
delta_in [8192] f32, bandwidth [1] f32) and returns (surv_func, surv_steps),
both [256, 8192] f32, matching reference.reference().

Strategy (8 NeuronCores, data-parallel over batch B=256 -> 32 rows/core):
  - per-core layout: partitions p = s*32 + b (s = one of 4 N-segments, b =
    local batch row), free axis f in [0,2048)
  - softmax / sum(p^2) / bandwidth scalars are computed on the HOST; the
    one-hot gather matrix H is host-built in fp8-e4m3 (chunk-major column
    order) and the gather term G = Phi.T @ H is a single mixed fp16xfp8
    matmul pass per (chunk, segment)
  - survival pipeline in log space with shifted-view algebra:
      v_n = (T - off - cumsum W)/T,  1 - shifted_n == v_{n-1}
      xi_n = ln(v_{n-1}) - ln(v_n)   (one Ln sweep, shifted AP views)
      mask = (lv > host-prepared per-element threshold) applied by
      multiply (lv made NaN-free via a 1e-6 pad inside the Ln bias)
      hz = cumsum(xi_masked), surv = exp(-hz)
  - cross-segment offsets via tiny constant-mask matmuls on the PE;
    steps/segment boundaries collapse via preset v_0 = surv_0 = 1 columns
  - outputs written fp16 and upcast on host
"""
import os
import sys

import numpy as np

for _p in ("/opt/trn_rl_repo", os.path.expanduser("~/.axon_site/_ro/trn_rl_repo")):
    if os.path.isdir(_p) and _p not in sys.path:
        sys.path.insert(0, _p)

import ml_dtypes

import concourse.bacc as bacc
import concourse.bass as bass
import concourse.mybir as mybir
import concourse.tile as tile

f32 = mybir.dt.float32
bf16 = mybir.dt.bfloat16
fp16 = mybir.dt.float16
fp8 = mybir.dt.float8e4

C, B, K = 16, 256, 8
N = 8192
NCORES = 8
Bc = B // NCORES          # 32
S = 4
F = N // S                # 2048
CH = 512                  # chunk width
NCH = F // CH             # 4
CK = C * K                # 128

TOLF = np.float32(1e-8 + 1e-5)
C1MTOL = np.float32(1.0 - float(TOLF))
EPS = np.float32(1e-13)
TINY = np.float32(1e-6)
BNDS = [0, 512, 1024, 1536, 1856, 2048]   # pipeline chunk boundaries
Alu = mybir.AluOpType
Act = mybir.ActivationFunctionType


def _consts():
    si = np.arange(128) // 32
    bi = np.arange(128) % 32
    same_b = bi[:, None] == bi[None, :]
    M1 = (same_b & (si[:, None] < si[None, :])).astype(np.float32)
    M2 = same_b.astype(np.float32)
    MD = M2 - M1
    return np.concatenate([M1, M2, MD], axis=1)  # [128, 384]


def build_nc():
    from contextlib import ExitStack

    nc = bacc.Bacc()

    phi_d = nc.dram_tensor("phi", [CK, Bc], fp16, kind="ExternalInput")
    esc_d = nc.dram_tensor("esc", [128, 2], f32, kind="ExternalInput")
    H_d = nc.dram_tensor("H", [128, N], fp8, kind="ExternalInput")
    delta_d = nc.dram_tensor("delta", [N], fp16, kind="ExternalInput")
    surv_d = nc.dram_tensor("surv", [Bc, N], fp16, kind="ExternalOutput")
    steps_d = nc.dram_tensor("steps", [Bc, N], fp16, kind="ExternalOutput")

    call_d = nc.inline_tensor(_consts(), "constall")

    with tile.TileContext(nc) as tc, ExitStack() as ctx:
        cons = ctx.enter_context(tc.tile_pool(name="cons", bufs=1))
        bigp = ctx.enter_context(tc.tile_pool(name="bigp", bufs=1))
        smal = ctx.enter_context(tc.tile_pool(name="smal", bufs=1))
        gps = ctx.enter_context(tc.tile_pool(name="gps", bufs=1, space="PSUM"))
        sps = ctx.enter_context(tc.tile_pool(name="sps", bufs=3, space="PSUM"))

        dma = nc.sync.dma_start

        # ---- input DMAs. Sync (SP) queue + HWDGE carry ONLY cpT and the H
        # stream (HWDGE's 625ns/DMA serial overhead is the front
        # bottleneck); everything else rides the Pool SWDGE queue, ordered
        # so delta's transfer queues up after the last H piece.
        call_t = cons.tile([128, 384], f32, tag="call")
        M1 = call_t[:, 0:128]
        M2 = call_t[:, 128:256]
        MD = call_t[:, 256:384]

        Phi = smal.tile([CK, Bc], fp16, tag="Phi")
        nc.gpsimd.dma_start(out=Phi, in_=phi_d[:, :])
        esc = smal.tile([128, 2], f32, tag="esc")
        nc.gpsimd.dma_start(out=esc, in_=esc_d[:, :])
        escale = esc[:, 0:1]
        ebias = esc[:, 1:2]

        H = bigp.tile([128, N], fp8, tag="H")
        for j in range(S):
            dma(out=H[:, j * F:(j + 1) * F],
                in_=bass.AP(tensor=H_d, offset=j * F, ap=[[N, 128], [1, F]]))
        dlt = bigp.tile([128, F], fp16, tag="dlt")
        dma(out=dlt, in_=bass.AP(tensor=delta_d, offset=0,
                                 ap=[[F, S], [0, Bc], [1, F]]))
        dma(out=call_t, in_=call_d[:, :])

        warm = smal.tile([128, 256], fp16, tag="warm")
        nc.gpsimd.memset(warm[:, :], 1.0)
        scrA = smal.tile([128, 1], f32, tag="scrA")
        nc.scalar.activation(out=scrA, in_=warm[:, 0:1], func=Act.Exp)

        # ---- presets ----
        xim = bigp.tile([128, F], fp16, tag="xim")
        surv = bigp.tile([128, F + 1], f32, tag="surv")
        nc.gpsimd.memset(surv[:, 0:1], 1.0)

        # ---- PE warm-up: keep the tensor engine continuously busy from t~0
        # so the p-state ramp reaches full clock before the G matmuls.
        # Output goes to g_ps[0], which the real chunk-0 matmul overwrites.
        g_ps = [gps.tile([128, CH], f32, name=f"g{j}", tag=f"g{j}")
                for j in range(NCH)]
        for _ in range(7):
            nc.tensor.matmul(g_ps[0][0:16, 0:256], warm[:, 0:16],
                             warm[:, 0:256], start=True, stop=True)


        # ---- G matmuls + exp + weights cumsum ----
        weights = bigp.tile([128, F], f32, tag="weights")
        scanW = bigp.tile([128, F], f32, tag="scanW")
        for j in range(NCH):
            for s in range(S):
                hs = H[:, j * F + s * CH: j * F + (s + 1) * CH]
                nc.tensor.matmul(g_ps[j][s * Bc:(s + 1) * Bc, :], Phi, hs,
                                 start=True, stop=True,
                                 tile_position=(0, s * Bc))
        for j in range(NCH):
            c0, c1 = j * CH, (j + 1) * CH
            nc.scalar.activation(out=weights[:, c0:c1], in_=g_ps[j],
                                 func=Act.Exp, bias=ebias, scale=escale)
            nc.vector.tensor_tensor_scan(
                out=scanW[:, c0:c1],
                data0=weights[:, c0:c1], data1=weights[:, c0:c1],
                initial=0.0 if j == 0 else scanW[:, c0 - 1: c0],
                op0=Alu.add, op1=Alu.bypass)

        # ---- T-barrier scalars (MD = M2 - M1 gives T - off in one matmul)
        vdiff_ps = sps.tile([128, 1], f32, tag="sp")
        nc.tensor.matmul(vdiff_ps, MD, scanW[:, F - 1:F], start=True, stop=True)
        Tall_ps = sps.tile([128, 1], f32, tag="sp")
        nc.tensor.matmul(Tall_ps, M2, scanW[:, F - 1:F], start=True, stop=True)

        Ts = smal.tile([128, 1], f32, tag="Ts")
        nc.vector.tensor_scalar(out=Ts, in0=Tall_ps, scalar1=float(EPS),
                                scalar2=None, op0=Alu.max)
        invT = smal.tile([128, 1], f32, tag="invT")
        nc.vector.reciprocal(out=invT, in_=Ts)
        sneg = smal.tile([128, 1], f32, tag="sneg")
        nc.vector.tensor_scalar(out=sneg, in0=invT, scalar1=-1.0,
                                scalar2=None, op0=Alu.mult)
        biasv = smal.tile([128, 1], f32, tag="biasv")
        nc.vector.tensor_scalar(out=biasv, in0=vdiff_ps, scalar1=invT,
                                scalar2=float(TINY), op0=Alu.mult, op1=Alu.add)

        # lv tile: col 0 = ln(v before first elem of each segment)
        lv = bigp.tile([128, F + 1], f32, tag="lv")
        nc.scalar.activation(out=lv[:, 0:1], in_=biasv, func=Act.Ln)

        mhat = bigp.tile([128, F], fp16, tag="mhat")
        hz = bigp.tile([128, F], f32, tag="hz")
        xi = bigp.tile([128, F], fp16, tag="xi")
        steps_loc = bigp.tile([128, F], f32, tag="steps_loc")

        NP = len(BNDS) - 1

        # ---- main pipeline ----
        # ACT: all lv chunks first (exp chunks interleave later by deps)
        for j in range(NP):
            c0, c1 = BNDS[j], BNDS[j + 1]
            nc.scalar.activation(out=lv[:, c0 + 1:c1 + 1], in_=scanW[:, c0:c1],
                                 func=Act.Ln, bias=biasv, scale=sneg)
        for j in range(NP):
            c0, c1 = BNDS[j], BNDS[j + 1]
            xi_eng = nc.vector if j == 0 else nc.gpsimd
            xi_eng.tensor_tensor(out=xi[:, c0:c1], in0=lv[:, c0:c1],
                                 in1=lv[:, c0 + 1:c1 + 1], op=Alu.subtract)
            nc.vector.tensor_tensor(out=mhat[:, c0:c1],
                                    in0=lv[:, c0 + 1:c1 + 1],
                                    in1=dlt[:, c0:c1], op=Alu.is_gt)
            nc.vector.tensor_tensor(out=xim[:, c0:c1], in0=xi[:, c0:c1],
                                    in1=mhat[:, c0:c1], op=Alu.mult)
            with tc.high_priority():
                nc.vector.tensor_tensor_scan(
                    out=hz[:, c0:c1], data0=xim[:, c0:c1],
                    data1=xim[:, c0:c1],
                    initial=0.0 if j == 0 else hz[:, c0 - 1: c0],
                    op0=Alu.add, op1=Alu.bypass)
            nc.scalar.activation(out=surv[:, c0 + 1:c1 + 1], in_=hz[:, c0:c1],
                                 func=Act.Exp, scale=-1.0)
            st_eng = nc.vector if j == 4 else nc.gpsimd
            st_eng.tensor_tensor(out=steps_loc[:, c0:c1],
                                 in0=surv[:, c0:c1],
                                 in1=surv[:, c0 + 1:c1 + 1],
                                 op=Alu.subtract)

        # ---- end scalars: e2 (cross-segment surv product), s2 normalizer.
        # ln(surv_local_last) == -hz_last, so the offset matmuls read hz
        # directly (no Ln round-trip, starts before the last exp finishes).
        off2_ps = sps.tile([128, 1], f32, tag="sp")
        nc.tensor.matmul(off2_ps, M1, hz[:, F - 1:F], start=True, stop=True)
        glog_ps = sps.tile([128, 1], f32, tag="sp")
        nc.tensor.matmul(glog_ps, M2, hz[:, F - 1:F], start=True, stop=True)
        e2 = smal.tile([128, 1], f32, tag="e2")
        nc.scalar.activation(out=e2, in_=off2_ps, func=Act.Exp, scale=-1.0)
        gl = smal.tile([128, 1], f32, tag="gl")
        nc.scalar.activation(out=gl, in_=glog_ps, func=Act.Exp, scale=-1.0)

        s2 = smal.tile([128, 1], f32, tag="s2")
        nc.vector.tensor_scalar(out=s2, in0=gl, scalar1=-1.0, scalar2=1.0,
                                op0=Alu.mult, op1=Alu.add)
        s2s = smal.tile([128, 1], f32, tag="s2s")
        nc.vector.tensor_scalar(out=s2s, in0=s2, scalar1=float(EPS),
                                scalar2=None, op0=Alu.max)
        rs2 = smal.tile([128, 1], f32, tag="rs2")
        nc.vector.reciprocal(out=rs2, in_=s2s)
        mask2 = smal.tile([128, 1], f32, tag="mask2")
        nc.vector.tensor_scalar(out=mask2, in0=s2, scalar1=float(EPS),
                                scalar2=None, op0=Alu.is_ge)
        rs2m = smal.tile([128, 1], f32, tag="rs2m")
        nc.vector.tensor_tensor(out=rs2m, in0=rs2, in1=mask2, op=Alu.mult)
        scal = smal.tile([128, 1], f32, tag="scal")
        nc.vector.tensor_tensor(out=scal, in0=e2, in1=rs2m, op=Alu.mult)

        # ---- tail: scale + DMA out (fp16), spread over engines. All output
        # DMAs ride the sync queue (its SEQ is idle by now).
        surv_out = bigp.tile([128, F], fp16, tag="survout")
        steps_out = bigp.tile([128, F], fp16, tag="stepsout")
        sv_eng = {0: nc.vector, 1: nc.scalar, 2: nc.vector, 3: nc.scalar,
                  4: nc.vector}
        st_eng2 = {0: nc.vector, 1: nc.scalar, 2: nc.vector, 3: nc.scalar,
                   4: nc.vector}
        for j in range(NP):
            c0, c1 = BNDS[j], BNDS[j + 1]
            eng = sv_eng[j]
            if eng is nc.scalar:
                eng.mul(out=surv_out[:, c0:c1], in_=surv[:, c0 + 1:c1 + 1],
                        mul=e2)
            else:
                eng.tensor_scalar(out=surv_out[:, c0:c1],
                                  in0=surv[:, c0 + 1:c1 + 1], scalar1=e2,
                                  scalar2=None, op0=Alu.mult)
        for j in range(NP):
            c0, c1 = BNDS[j], BNDS[j + 1]
            eng = st_eng2[j]
            if eng is nc.scalar:
                eng.mul(out=steps_out[:, c0:c1], in_=steps_loc[:, c0:c1],
                        mul=scal)
            else:
                eng.tensor_scalar(out=steps_out[:, c0:c1],
                                  in0=steps_loc[:, c0:c1], scalar1=scal,
                                  scalar2=None, op0=Alu.mult)
        for o0 in (0, 1024):
            sv_dst = bass.AP(tensor=surv_d, offset=o0,
                             ap=[[F, S], [N, Bc], [1, 1024]])
            dma(out=sv_dst, in_=surv_out[:, o0:o0 + 1024])
        for o0 in (0, 1024):
            st_dst = bass.AP(tensor=steps_d, offset=o0,
                             ap=[[F, S], [N, Bc], [1, 1024]])
            dma(out=st_dst, in_=steps_out[:, o0:o0 + 1024])

    # Prefer the activation table containing Exp, Ln, Copy etc. so the whole
    # kernel needs a single table load.
    import concourse.bacc as _bacc_mod
    import concourse.hw_specs as _hw
    _orig_get = _hw.get_activation_tables

    def _filtered(arch):
        t = dict(_orig_get(arch))
        pref = [k for k in t if "natural_log_exp" in k]
        if not pref:
            return t
        mine = {f for f in t[pref[0]]
                if getattr(f, "name", str(f)) in ("Exp", "Ln", "Copy",
                                                  "Identity")}
        out = {}
        for k, fns in t.items():
            out[k] = set(fns) if k in pref else set(fns) - mine
        return out

    _bacc_mod.get_activation_tables = _filtered
    try:
        nc.compile()
    finally:
        _bacc_mod.get_activation_tables = _orig_get
    return nc


def make_in_maps(c_p, c_in, delta_in, bandwidth):
    c_p = np.asarray(c_p, np.float32)
    c_in = np.asarray(c_in)
    delta_in = np.asarray(delta_in, np.float32)
    bandwidth = np.asarray(bandwidth, np.float32)

    # one-hot H [p = k*16 + c, n], chunk-major columns:
    # column j*2048 + s*512 + i  <->  global n = s*2048 + j*512 + i
    ks = (np.arange(128) // 16).astype(c_in.dtype)          # [128]
    cs = np.arange(128) % 16                                 # [128]
    Hfull = (c_in[:, cs].T == ks[:, None])                   # [128, N] bool
    Hperm = (Hfull.reshape(128, S, NCH, CH)
             .transpose(0, 2, 1, 3)
             .reshape(128, N)).astype(ml_dtypes.float8_e4m3)

    LTOL = float(np.log(np.float64(TOLF) + 1e-6))
    delta_bf = np.where(delta_in > 0.5, np.float32(LTOL),
                        np.float32(60000.0)).astype(np.float16)
    band = bandwidth.reshape(1, 1)

    in_maps = []
    for core in range(NCORES):
        b0 = core * Bc
        cp_local = c_p[:, b0:b0 + Bc, :].astype(np.float64)  # [C, Bc, K]
        e = np.exp(cp_local)
        p = e / e.sum(axis=-1, keepdims=True)                # softmax [C,Bc,K]
        phi = np.ascontiguousarray(
            p.transpose(2, 0, 1).reshape(CK, Bc)).astype(np.float16)
        A = (p * p).sum(axis=-1).sum(axis=0)                 # [Bc]
        bw = float(np.clip(band[0, 0], 0.1, 10.0))
        esc = np.empty((128, 2), np.float32)
        esc[:, 0] = 2.0 / bw                                 # escale
        esc[:, 1] = np.tile(-(A + 16.0) / bw, S)             # ebias
        in_maps.append({"phi": phi, "esc": esc, "H": Hperm,
                        "delta": delta_bf})
    return in_maps


_CACHED_NC = None
_CACHED_RUN = None


def _get_nc():
    global _CACHED_NC
    if _CACHED_NC is None:
        _CACHED_NC = build_nc()
    return _CACHED_NC


def _get_runner():
    """Build (once) a cached sharded jit callable over the 8 cores."""
    global _CACHED_RUN
    if _CACHED_RUN is not None:
        return _CACHED_RUN
    import jax
    from jax.sharding import Mesh, PartitionSpec
    from jax.experimental.shard_map import shard_map
    import concourse.mybir as mb
    from concourse import bass2jax
    from concourse.bass2jax import (_bass_exec_p, install_neuronx_cc_hook,
                                    partition_id_tensor)

    nc = _get_nc()
    install_neuronx_cc_hook()

    pid_name = nc.partition_id_tensor.name if nc.partition_id_tensor else None
    in_names, out_names, out_avals, zero_shapes = [], [], [], []
    for alloc in nc.m.functions[0].allocations:
        if not isinstance(alloc, mb.MemoryLocationSet):
            continue
        if not alloc.memorylocations:
            continue
        name = alloc.memorylocations[0].name
        if alloc.kind == "ExternalInput":
            if name == pid_name:
                continue
            in_names.append(name)
        elif alloc.kind == "ExternalOutput":
            out_names.append(name)
            shape = tuple(alloc.tensor_shape)
            dtype = mb.dt.np(alloc.dtype)
            out_avals.append(jax.core.ShapedArray(shape, dtype))
            zero_shapes.append((shape, dtype))
    n_params = len(in_names)
    all_names = in_names + out_names
    if pid_name is not None:
        all_names = all_names + [pid_name]
    donate = tuple(range(n_params, n_params + len(out_names)))

    def _body(*args):
        operands = list(args)
        if pid_name is not None:
            operands.append(partition_id_tensor())
        outs = _bass_exec_p.bind(
            *operands, out_avals=tuple(out_avals), in_names=tuple(all_names),
            out_names=tuple(out_names), lowering_input_output_aliases=(),
            sim_require_finite=False, sim_require_nnan=False, nc=nc)
        return tuple(outs)

    devices = jax.devices()[:NCORES]
    mesh = Mesh(np.asarray(devices), ("core",))
    specs = (PartitionSpec("core"),) * (n_params + len(out_names))
    out_specs = (PartitionSpec("core"),) * len(out_names)
    sharded = jax.jit(
        shard_map(_body, mesh=mesh, in_specs=specs, out_specs=out_specs,
                  check_rep=False),
        donate_argnums=donate, keep_unused=True)

    def run(in_maps):
        concat_in = [
            np.concatenate([np.asarray(im[name]) for im in in_maps], axis=0)
            for name in in_names]
        concat_zeros = [
            np.zeros((NCORES * sh[0], *sh[1:]), dt) for sh, dt in zero_shapes]
        out = sharded(*concat_in, *concat_zeros)
        res = {}
        for i, name in enumerate(out_names):
            res[name] = np.asarray(out[i])  # [NCORES*Bc, N]
        return res

    _CACHED_RUN = run
    return run


def kernel(c_p, c_in, delta_in, bandwidth):
    in_maps = make_in_maps(c_p, c_in, delta_in, bandwidth)
    run = _get_runner()
    res = run(in_maps)
    return res["surv"].astype(np.float32), res["steps"].astype(np.float32)


if __name__ == "__main__":
    rng = np.random.default_rng(0)
    c_p = rng.standard_normal((C, B, K), dtype=np.float32)
    c_in = rng.integers(0, K, size=(N, C)).astype(np.int32)
    delta = (rng.random(N) > 0.3).astype(np.float32)
    band = np.ones((1,), np.float32)
    import time
    t0 = time.time()
    sf, ss = kernel(c_p=c_p, c_in=c_in, delta_in=delta, bandwidth=band)
    print("first call", time.time() - t0, "s", sf.shape, ss.shape,
          float(sf.sum()), float(ss.sum()))
    t0 = time.time()
    sf, ss = kernel(c_p=c_p, c_in=c_in, delta_in=delta, bandwidth=band)
    print("second call", time.time() - t0, "s")
